# revision 16
# baseline (speedup 1.0000x reference)
"""Trainium2 Bass kernel for nn_Averager (pooling, 3-level box-average).

Math (verified vs reference): per sample, with input x[n, i, c] where
n = (n5 n4 n3 n2 n1 n0) base-4 digits, c = (c2 c1 c0) base-4 digits:
  out[:, :, 0, :] = x[:, :, 0, :]
  out1[n, c] = E[n4, n2, c2, c0, n0, c1],
      E[r5, r4, r3, r0; g2, g1] = mean over (n2, n1, c0) of x1
  out2[n, c] = G[c2, c1, c0],
      G[p, q, r] = mean over (n4, n3, n1, n0, c1, c0) of x2 with
      (n5, c2in, n2) = (p, q, r)

Sharding: data-parallel over batch, 4 samples per core on 8 cores,
processed as 2 groups of 2 samples.

Layout (pair-contiguous): SBUF partition p = b*64 + n//64 =
(b, n5, n4, n3); free j = n % 64 = 16*n2 + 4*n1 + n0, row (i, c).
A 6MB group is contiguous in DRAM and per-partition contiguous in SBUF:
each group is ONE 2-D in-DMA pair (halves, so compute starts when the
first half lands) and ONE contiguous [128 x 48KB] out-DMA (every
packet is >=16KB-class; sub-512B packets would eat the SDMA
read-modify-write penalty).

Ring assignment (measured): SWDGE/gpsimd sustains ~412 GB/s for the
DRAM->SBUF loads, so all four x in-halves ride it back-to-back; the
tiny s12 constant rides the scalar HWDGE ring in parallel so it never
delays the x stream.  The two 6MB outs ride the scalar (g0) and sync
(g1) HWDGE rings (~428 GB/s SBUF->DRAM) and overlap the in stream.

Compute: lane-local reductions on DVE, cross-partition routing on the
PE in bf16 (selector values 1/64 and 1/4096 are exact in bf16;
stage-A sums carry ~2^-9 relative error, well inside the 2e-2
budget).  Per group: u0 and the first L2 half-reduce (A2ra) run off
in-half 1; A2rb, then the u1/w/h1/h2/A tree off half 2 — so the L2
selector matmuls fire early and the ACT engine broadcasts the
64-float G row to all 64 j-rows straight out of PSUM while DVE works
the L1 tree.  L1 evacuation is 4 PSUM->SBUF copies whose source AP
broadcasts over n1 (replication during the read), split 2 on DVE + 2
on ACT so the post-matmul wall is ~2.4us.  Outputs are assembled
IN-PLACE in the input tile: L0 is untouched input, L1 the evacuation,
L2 the ACT broadcast.
"""

import numpy as np

N_CORES = 8
B_FULL = 32
B_CORE = B_FULL // N_CORES  # 4
N = 4096
LVL = 3
C = 64


def _make_selectors():
    """Routing selectors, pair layout: k = 64*b + 16*k5 + 4*k4 + k3.

    S1 block (n2o, c2o), 16 blocks:
        S1[k, m] = 1/64   iff b(k)==b(m), k5==m4, k4==n2o, k3==c2o
    S2 block (c2o), 4 blocks:
        S2[k, m] = 1/4096 iff b(k)==b(m), k5==c2o

    Returned in bfloat16 (both scale factors are powers of two, exact).
    """
    import ml_dtypes

    k = np.arange(128)
    b, k5, k4, k3 = k >> 6, (k >> 4) & 3, (k >> 2) & 3, k & 3
    m = np.arange(128)
    bm, m4 = m >> 6, (m >> 2) & 3
    S1 = np.zeros((128, 16, 128), np.float32)
    S2 = np.zeros((128, 4, 128), np.float32)
    for n2o in range(4):
        for c2o in range(4):
            S1[:, n2o * 4 + c2o, :] = (
                (b[:, None] == bm[None, :])
                & (k5[:, None] == m4[None, :])
                & (k4[:, None] == n2o)
                & (k3[:, None] == c2o)
            ).astype(np.float32) / 64.0
    for c2o in range(4):
        S2[:, c2o, :] = (
            (b[:, None] == bm[None, :]) & (k5[:, None] == c2o)
        ).astype(np.float32) / 4096.0
    bf16 = ml_dtypes.bfloat16
    return (
        np.ascontiguousarray(S1.reshape(128, 2048).astype(bf16)),
        np.ascontiguousarray(S2.reshape(128, 512).astype(bf16)),
    )


def _build_nc():
    import concourse.bass as bass
    import concourse.tile as tile
    from concourse import mybir

    dt = mybir.dt.float32
    bt = mybir.dt.bfloat16
    X = mybir.AxisListType.X
    XY = mybir.AxisListType.XY
    ADD = mybir.AluOpType.add

    from concourse import bacc
    nc = bacc.Bacc()
    x = nc.declare_dram_parameter("x", [B_CORE, N, LVL, C], dt, isOutput=False)
    s12 = nc.declare_dram_parameter("s12", [128, 2560], bt, isOutput=False)
    out = nc.declare_dram_parameter("out", [B_CORE, N, LVL, C], dt, isOutput=True)

    with tile.TileContext(nc) as tc:
        with (
            tc.tile_pool(name="consts", bufs=1) as cpool,
            tc.tile_pool(name="xin", bufs=2) as xpool,
            tc.tile_pool(name="tmp", bufs=1) as tpool,
            tc.tile_pool(name="psum", bufs=2, space="PSUM") as ppool,
        ):
            # ---- pre-issue every input load ----
            # x halves ride SWDGE back-to-back (FIFO, sequential
            # arrival); s12 rides the scalar HWDGE ring in parallel.
            s12sb = cpool.tile([128, 2560], bt, tag="s12")
            nc.scalar.dma_start(s12sb[:], s12[:])
            xts = []
            for g in range(B_CORE // 2):
                xt = xpool.tile([128, 12288], dt, tag="xt")
                xsrc = x[slice(2 * g, 2 * g + 2)].rearrange(
                    "b (ph j) i c -> (b ph) (j i c)", ph=64
                )
                # all four halves ride SWDGE back-to-back: one FIFO ring
                # keeps arrival strictly sequential at ~412 GB/s
                nc.gpsimd.dma_start(xt[:, 0:6144], xsrc[:, 0:6144])
                nc.gpsimd.dma_start(xt[:, 6144:12288], xsrc[:, 6144:12288])
                xts.append(xt)
            s1sb = s12sb[:, 0:2048]
            s2sb = s12sb[:, 2048:2560]

            for g in range(B_CORE // 2):
                bs = slice(2 * g, 2 * g + 2)
                xt = xts[g]
                xtv = xt[:].rearrange(
                    "p (j i c) -> p j i c", j=64, i=3, c=64
                )

                # ---- L1 stage A, half-1 side: u0 = n2_0 + n2_1, then
                # reduce c0 and n1 down to the 64-elem partial P0 — all
                # of it runs while half 2 is still in flight ----
                v = xt[:].rearrange(
                    "p (n2 n1 n0 i c) -> p n2 n1 n0 i c",
                    n2=4, n1=4, n0=4, i=3, c=64,
                )
                u0 = tpool.tile([128, 1024], dt, tag="u0")
                nc.vector.tensor_add(
                    u0[:].rearrange("p (n1 n0 c) -> p n1 n0 c", n1=4, n0=4, c=64),
                    v[:, 0, :, :, 1, :], v[:, 1, :, :, 1, :],
                )
                r0 = tpool.tile([128, 256], dt, tag="r0")
                nc.vector.tensor_reduce(
                    r0[:].rearrange(
                        "p (n1 c1 n0 c2) -> p n1 n0 c2 c1", n1=4, c1=4, n0=4, c2=4
                    ),
                    u0[:].rearrange("p (k c0) -> p k c0", k=256, c0=4),
                    axis=X, op=ADD,
                )
                P0 = tpool.tile([128, 64], dt, tag="P0")
                nc.vector.tensor_reduce(
                    P0[:],
                    r0[:].rearrange("p (n1 k) -> p k n1", n1=4, k=64),
                    axis=X, op=ADD,
                )

                # ---- L2 stage A: two XY-reduces over (nn, cc), one per
                # in-half, so each runs as soon as its half lands ----
                # A2 free = 4*c2 + n2; in free = [n2, c2, nn, cc]
                A2 = tpool.tile([128, 16], dt, tag="A2")
                A2v = A2[:].rearrange("p (c2 n2) -> p n2 c2", c2=4, n2=4)
                xl2 = xt[:].rearrange(
                    "p (n2 nn i c2 cc) -> p n2 i c2 nn cc",
                    n2=4, nn=16, i=3, c2=4, cc=16,
                )
                nc.vector.tensor_reduce(
                    A2v[:, 0:2, :], xl2[:, 0:2, 2, :, :, :], axis=XY, op=ADD,
                )
                nc.vector.tensor_reduce(
                    A2v[:, 2:4, :], xl2[:, 2:4, 2, :, :, :], axis=XY, op=ADD,
                )
                A2b = tpool.tile([128, 16], bt, tag="A2b")
                nc.vector.tensor_copy(A2b[:], A2[:])

                # ---- L2: 4 reduce+broadcast matmuls -> gp psum ----
                # gp free = 16*c2o + (4*c1o + c0o); rhs j = (c2in, n2)
                gp = ppool.tile([128, 64], dt, tag="gp")
                for c2o in range(4):
                    nc.tensor.matmul(
                        gp[:, c2o * 16:(c2o + 1) * 16],
                        s2sb[:, c2o * 128:(c2o + 1) * 128],
                        A2b[:, 0:16],
                        start=True, stop=True,
                    )
                # ---- L2 evac: ACT broadcasts the 64-float G row to all
                # 64 j-rows straight out of PSUM while DVE runs the L1
                # tree (WAR on the level-2 lanes: waits on the A2 reduces)
                nc.scalar.copy(
                    xtv[:, :, 2, :],
                    gp[:].rearrange("p (o c) -> p o c", o=1, c=64)
                    .broadcast_to((128, 64, 64)),
                )

                # ---- L1 stage A, half-2 side: u1, reduce to P1, then
                # A = P0 + P1.  A's free layout puts the out-j digit at
                # stride 1 (f = 16*c1 + 4*n0 + c2) so the c1p PSUM
                # blocks merge with the contiguous out-c dim and the
                # whole L1 evac becomes ONE copy ----
                u1 = tpool.tile([128, 1024], dt, tag="u1")
                nc.vector.tensor_add(
                    u1[:].rearrange("p (n1 n0 c) -> p n1 n0 c", n1=4, n0=4, c=64),
                    v[:, 2, :, :, 1, :], v[:, 3, :, :, 1, :],
                )
                r1 = tpool.tile([128, 256], dt, tag="r1")
                nc.vector.tensor_reduce(
                    r1[:].rearrange(
                        "p (n1 c1 n0 c2) -> p n1 n0 c2 c1", n1=4, c1=4, n0=4, c2=4
                    ),
                    u1[:].rearrange("p (k c0) -> p k c0", k=256, c0=4),
                    axis=X, op=ADD,
                )
                A = tpool.tile([128, 64], dt, tag="A")
                nc.vector.tensor_reduce(
                    A[:],
                    r1[:].rearrange("p (n1 k) -> p k n1", n1=4, k=64),
                    axis=X, op=ADD,
                )
                nc.vector.tensor_add(A[:], A[:], P0[:])
                Ab = tpool.tile([128, 64], bt, tag="Ab")
                nc.vector.tensor_copy(Ab[:], A[:])

                # ---- L1: 16 routing matmuls -> c1p psum (2 banks) ----
                # c1p free = 64*(4*n2o + c2o) + (16*c1 + 4*c0 + n0-ish)
                c1p = ppool.tile([128, 1024], dt, tag="c1p")
                for n2o in range(4):
                    for c2o in range(4):
                        blk = n2o * 4 + c2o
                        nc.tensor.matmul(
                            c1p[:, blk * 64:(blk + 1) * 64],
                            s1sb[:, blk * 128:(blk + 1) * 128],
                            Ab[:, 0:64],
                            start=True, stop=True,
                        )

                # ---- L1 evac: ONE broadcast copy.  With Ab's layout,
                # c1p free = 256*n2 + 4*(out-c) + n0, so (c2o-block,
                # within-block c) merge into a single stride-4 dim of
                # 64 and the n1 replication is a stride-0 dim: the full
                # 4096-elem evacuation is a single 4-dim DVE copy ----
                xte = xt[:].rearrange(
                    "p (n2 n1 n0 i c) -> p n2 n1 n0 i c",
                    n2=4, n1=4, n0=4, i=3, c=64,
                )
                nc.vector.tensor_copy(
                    xte[:, :, :, :, 1, :],
                    c1p[:].rearrange(
                        "p (n2 o c n0) -> p n2 o n0 c", n2=4, o=1, c=64, n0=4
                    ).broadcast_to((128, 4, 4, 4, 64)),
                )

                # ---- out: ONE contiguous DMA per group ----
                # g0 rides the scalar ring (empty after s12), g1 the
                # sync ring (idle), so the two 6MB outs drain on
                # separate rings and overlap the in stream.
                outv = out[bs].rearrange("b (ph j) i c -> (b ph) (j i c)", ph=64)
                hw = nc.scalar if g == 0 else nc.sync
                hw.dma_start(outv[:, :], xt[:, :])
    nc.compile()
    return nc


_NC_CACHE = {}


def _get_nc():
    if "nc" not in _NC_CACHE:
        _NC_CACHE["nc"] = _build_nc()
    return _NC_CACHE["nc"]


def kernel(**inputs: np.ndarray) -> np.ndarray:
    from concourse.bass_utils import run_bass_kernel_spmd

    x = np.ascontiguousarray(inputs["x"], dtype=np.float32)
    assert x.shape == (B_FULL, N, LVL, C), x.shape
    S1, S2 = _make_selectors()
    S12 = np.ascontiguousarray(np.concatenate([S1, S2], axis=1))
    nc = _get_nc()
    in_maps = [
        {"x": np.ascontiguousarray(x[k * B_CORE:(k + 1) * B_CORE]),
         "s12": S12}
        for k in range(N_CORES)
    ]
    res = run_bass_kernel_spmd(nc, in_maps, list(range(N_CORES)))
    outs = [res.results[k]["out"] for k in range(N_CORES)]
    return np.ascontiguousarray(np.concatenate(outs, axis=0))
